# revision 12
# baseline (speedup 1.0000x reference)
"""BertSelfAttention TRN2 Bass kernel (8-core data-parallel over batch).

Per core (one batch element), per head:
  qk projection -> q,k in SBUF (biases folded in via rank-1 ones-row matmuls)
  stats pass:  scores in [q-part, k-free] orientation (mask folded via aux
               row) -> fused elementwise-max + reduce-max on DVE gives the
               exact per-query max m_q in one op per q-tile
  main pass:   scores in [k-part, q-free] orientation with two aux rows
               (mask, and -1 paired against a c-row holding +m_q, so the
               matmul itself computes s - m) -> single exp on ACT -> bf16
               probs e8
  context:     out[q, d] orientation: stationary e8 [k,q-tile], moving
               v_aug [k, d|1] (bf16) accumulated over k tiles; the 65th
               column of v_aug is 1 so the same matmul yields Z; normalize
               by 1/Z straight out of PSUM.
All f32 matmuls run in float32r (fast PE mode, fp32 PSUM accumulation).
"""
import sys

sys.path.insert(0, "/opt/trn_rl_repo")

import numpy as np
import concourse.bacc as bacc
import concourse.mybir as mybir
import concourse.tile as tile
from concourse.bass_utils import run_bass_kernel_spmd

F32 = mybir.dt.float32
F32R = mybir.dt.float32r
BF16 = mybir.dt.bfloat16
EXP = mybir.ActivationFunctionType.Exp
MAX = mybir.AluOpType.max

HD = 64  # head dim (fixed)


def build_module(T, H, NH):
    """One-core program; run SPMD on 8 cores with per-core batch slices."""
    NT = T // 128      # token tiles
    NHT = H // 128     # hidden-dim tiles
    QC = min(512, T)   # moving chunk (>=256 keeps f32r at full rate)
    NQC = T // QC

    nc = bacc.Bacc("TRN2", target_bir_lowering=False, debug=False, num_devices=8)

    hidden = nc.dram_tensor("hidden", [T, H], F32R, kind="ExternalInput").ap()
    w = nc.dram_tensor("w", [H, 3 * H], F32R, kind="ExternalInput").ap()
    mask_neg = nc.dram_tensor("mask_neg", [1, T], F32R, kind="ExternalInput").ap()
    ones_row = nc.dram_tensor("ones_row", [1, T], F32R, kind="ExternalInput").ap()
    neg_row = nc.dram_tensor("neg_row", [1, T], F32R, kind="ExternalInput").ap()
    qkb = nc.dram_tensor("qkb", [1, 128 * NH], F32R, kind="ExternalInput").ap()
    vb = nc.dram_tensor("vb", [1, H], F32R, kind="ExternalInput").ap()
    ident_r = nc.dram_tensor("ident_r", [128, 128], F32R, kind="ExternalInput").ap()
    out = nc.dram_tensor("out", [T, H], F32, kind="ExternalOutput").ap()

    out_r = out.rearrange("(qt p) (h d) -> p qt h d", p=128, d=HD)

    with tile.TileContext(nc) as tc:
        with tc.tile_pool(name="persist", bufs=1) as persist, tc.tile_pool(
            name="work", bufs=2
        ) as work, tc.tile_pool(name="e8p", bufs=12) as e8p, tc.tile_pool(
            name="outp", bufs=2
        ) as outp, tc.tile_pool(name="rzp", bufs=2) as rzp, tc.tile_pool(
            name="psa", bufs=1, space="PSUM"
        ) as psa, tc.tile_pool(name="pst", bufs=1, space="PSUM") as pst, tc.tile_pool(
            name="psp", bufs=2, space="PSUM"
        ) as psp, tc.tile_pool(name="psc", bufs=1, space="PSUM") as psc:
            # ---- static constants ----
            idr = persist.tile([128, 128], F32R, tag="idr")
            nc.sync.dma_start(out=idr, in_=ident_r)
            onesr = persist.tile([1, T], F32R, tag="onesr")
            nc.sync.dma_start(out=onesr, in_=ones_row)
            qkbt = persist.tile([1, 128 * NH], F32R, tag="qkbt")
            nc.sync.dma_start(out=qkbt, in_=qkb)
            vbt = persist.tile([1, H], F32R, tag="vbt")
            nc.sync.dma_start(out=vbt, in_=vb)

            # persistent q/k aux buffers:
            # qaux rows: 0:64 q, 64 ones (static), 65 c=+max (per head)
            # kaux rows: 0:64 k, 64 mask*-1e4 (static), 65 -1 (static)
            qaux = persist.tile([66, T], F32R, tag="qaux")
            kaux = persist.tile([66, T], F32R, tag="kaux")
            nc.sync.dma_start(out=qaux[64:65, :], in_=ones_row)
            nc.sync.dma_start(out=kaux[64:65, :], in_=mask_neg)
            nc.sync.dma_start(out=kaux[65:66, :], in_=neg_row)

            # ---- phase 0: hT[p, ht, t] = hidden[t, ht*128+p] ----
            hT = persist.tile([128, NHT, T], F32R, tag="hT")
            for t in range(NT):
                hid = work.tile([128, H], F32R, tag="hid")
                nc.sync.dma_start(out=hid, in_=hidden[t * 128 : (t + 1) * 128, :])
                pool = psa if t % 2 == 0 else pst
                xpt = pool.tile([128, NHT, 128], F32R, tag="pj")
                for hb in range(NHT):
                    nc.tensor.transpose(
                        xpt[:, hb, :], hid[:, hb * 128 : (hb + 1) * 128], idr[:]
                    )
                nc.vector.tensor_copy(hT[:, :, t * 128 : (t + 1) * 128], xpt[:])

            # ---- phase 1: v_aug[p, kt, h, 0:64] = v proj + bias; [.., 64] = 1 ----
            wv = persist.tile([128, NHT, H], F32R, tag="wv")
            for ht in range(NHT):
                wsl = w[ht * 128 : (ht + 1) * 128, :].rearrange(
                    "p (h three d) -> p h three d", three=3, d=HD
                )
                nc.sync.dma_start(
                    out=wv[:, ht, :].rearrange("p (h d) -> p h d", d=HD),
                    in_=wsl[:, :, 2, :],
                )
            v_aug = persist.tile([128, NT, NH, HD + 1], BF16, tag="v_aug")
            nc.vector.memset(v_aug[:, :, :, HD : HD + 1], 1.0)
            VW = min(512, H)
            NVH = VW // HD
            for t in range(NT):
                for half in range(H // VW):
                    vp = psp.tile([128, VW], F32, tag="sp")  # 512-wide, 1 bank
                    for ht in range(NHT):
                        nc.tensor.matmul(
                            vp[:],
                            hT[:, ht, t * 128 : (t + 1) * 128],
                            wv[:, ht, half * VW : (half + 1) * VW],
                            start=(ht == 0),
                            stop=False,
                        )
                    # bias: out[t, f] += 1 * vb[f]
                    nc.tensor.matmul(
                        vp[:],
                        onesr[0:1, t * 128 : (t + 1) * 128],
                        vbt[0:1, half * VW : (half + 1) * VW],
                        start=False,
                        stop=True,
                    )
                    nc.vector.tensor_copy(
                        v_aug[:, t, half * NVH : (half + 1) * NVH, 0:HD],
                        vp[:].rearrange("p (h d) -> p h d", d=HD),
                    )

            # ---- per-head attention ----
            for h in range(NH):
                # fused q|k projection for this head -> psum [128(q0:64,k64:128), T]
                wqk = work.tile([128, NHT, 128], F32R, tag="wqk")
                nc.sync.dma_start(
                    out=wqk,
                    in_=w[:, h * 3 * HD : h * 3 * HD + 128].rearrange(
                        "(ht p) f -> p ht f", p=128
                    ),
                )
                qkp = psa.tile([128, T], F32, tag="pj")  # proj-only pool
                for qc in range(NQC):
                    for ht in range(NHT):
                        nc.tensor.matmul(
                            qkp[:, qc * QC : (qc + 1) * QC],
                            wqk[:, ht, :],
                            hT[:, ht, qc * QC : (qc + 1) * QC],
                            start=(ht == 0),
                            stop=False,
                        )
                    # bias: out[f, t] += qkb[f] * 1
                    nc.tensor.matmul(
                        qkp[:, qc * QC : (qc + 1) * QC],
                        qkbt[0:1, h * 128 : (h + 1) * 128],
                        onesr[0:1, qc * QC : (qc + 1) * QC],
                        start=False,
                        stop=True,
                    )
                nc.vector.tensor_copy(qaux[0:64, :], qkp[0:64, :])
                nc.vector.tensor_copy(kaux[0:64, :], qkp[64:128, :])

                # stats pass: per q-tile max of masked raw scores
                cmat = persist.tile([128, NT], F32R, tag="cmat")
                for qt in range(NT):
                    smx = pst.tile([128, T], F32, tag="pj")
                    for qc in range(NQC):
                        nc.tensor.matmul(
                            smx[:, qc * QC : (qc + 1) * QC],
                            qaux[0:65, qt * 128 : (qt + 1) * 128],
                            kaux[0:65, qc * QC : (qc + 1) * QC],
                            start=True,
                            stop=True,
                        )
                    nc.vector.reduce_max(
                        cmat[:, qt : qt + 1], smx[:], axis=mybir.AxisListType.X
                    )
                # c row: transpose [128, NT] -> [NT, 128] -> qaux row 65
                ctile = psp.tile([NT, 128], F32R, tag="sp")
                nc.tensor.transpose(ctile[:], cmat[:], idr[:])
                ctr = work.tile([NT, 128], F32R, tag="ctr")
                nc.vector.tensor_copy(ctr[:], ctile[:])
                nc.sync.dma_start(out=qaux[65:66, :], in_=ctr[:])

                # main pass: scores with mask and -max folded in -> exp -> bf16
                e8s = []
                for kt in range(NT):
                    e8 = e8p.tile([128, T], BF16, tag="e8")
                    for qc in range(NQC):
                        sp = psp.tile([128, QC], F32, tag="sp")
                        nc.tensor.matmul(
                            sp[:],
                            kaux[0:66, kt * 128 : (kt + 1) * 128],
                            qaux[0:66, qc * QC : (qc + 1) * QC],
                            start=True,
                            stop=True,
                        )
                        nc.scalar.activation(
                            out=e8[:, qc * QC : (qc + 1) * QC],
                            in_=sp[:],
                            func=EXP,
                            scale=8.0,
                        )
                    e8s.append(e8)

                # context: out[q, d|Z] accumulated over k tiles
                ctq = psc.tile([128, NT, 128], F32, tag="cq")
                for qt in range(NT):
                    for kt in range(NT):
                        nc.tensor.matmul(
                            ctq[:, qt, 0 : HD + 1],
                            e8s[kt][:, qt * 128 : (qt + 1) * 128],
                            v_aug[:, kt, h, :],
                            start=(kt == 0),
                            stop=(kt == NT - 1),
                        )
                # normalize and store
                ost = outp.tile([128, NT, HD], F32, tag="ost")
                for qt in range(NT):
                    rz = rzp.tile([128, 1], F32, tag="rz")
                    nc.vector.reciprocal(rz[:], ctq[:, qt, HD : HD + 1])
                    nc.vector.tensor_scalar_mul(
                        ost[:, qt, :], ctq[:, qt, 0:HD], rz[:]
                    )
                nc.sync.dma_start(out=out_r[:, :, h, :], in_=ost[:])

    nc.compile()
    return nc


_module_cache = {}


def _get_module(T, H, NH):
    key = (T, H, NH)
    if key not in _module_cache:
        _module_cache[key] = build_module(T, H, NH)
    return _module_cache[key]


def run_sharded(hidden_states, attention_mask, w_qkv, b_qkv, trace=False):
    B, T, H = hidden_states.shape
    NH = H // HD
    nc = _get_module(T, H, NH)

    w_np = np.ascontiguousarray(w_qkv.astype(np.float32))
    b_np = np.asarray(b_qkv, dtype=np.float32)
    # qkb[h*128 + p] = b[h*192 + p]  (q bias 0:64, k bias 64:128 per head)
    qkb = np.empty((1, 128 * NH), np.float32)
    vb = np.empty((1, H), np.float32)
    for h in range(NH):
        qkb[0, h * 128 : (h + 1) * 128] = b_np[h * 3 * HD : h * 3 * HD + 128]
        vb[0, h * HD : (h + 1) * HD] = b_np[h * 3 * HD + 2 * HD : h * 3 * HD + 3 * HD]
    ones_row = np.ones((1, T), np.float32)
    neg_row = np.full((1, T), -1.0, np.float32)
    ident = np.eye(128, dtype=np.float32)

    in_maps = []
    for b in range(B):
        m = np.asarray(attention_mask[b]).reshape(-1).astype(np.float32)
        in_maps.append(
            dict(
                hidden=np.ascontiguousarray(hidden_states[b].astype(np.float32)),
                w=w_np,
                mask_neg=(m * np.float32(-10000.0)).reshape(1, T),
                ones_row=ones_row,
                neg_row=neg_row,
                qkb=qkb,
                vb=vb,
                ident_r=ident,
            )
        )
    res = run_bass_kernel_spmd(nc, in_maps, core_ids=list(range(B)), trace=trace)
    return np.stack([res.results[b]["out"] for b in range(B)]), res


def kernel(hidden_states, attention_mask, w_qkv, b_qkv):
    out, _ = run_sharded(
        np.asarray(hidden_states),
        np.asarray(attention_mask),
        np.asarray(w_qkv),
        np.asarray(b_qkv),
    )
    return out.astype(np.float32)


# revision 17
# speedup vs baseline: 1.4559x; 1.4559x over previous
"""BertSelfAttention TRN2 Bass kernel (8-core data-parallel over batch).

Per core (one batch element), per head:
  qk projection -> q,k in SBUF (biases folded in via rank-1 ones-row matmuls)
  stats pass:  scores in [q-part, k-free] orientation (mask folded via aux
               row) -> per-query max m_q via DVE reduce_max
  main pass:   scores in [k-part, q-free] orientation with two aux rows
               (mask, and -1 paired against a c-row holding +m_q, so the
               matmul itself computes s - m) -> single exp on ACT -> bf16
               probs e8
  context:     out[q, d] orientation: stationary e8 [k,q-tile], moving
               v_aug [k, d|1] (bf16) accumulated over k tiles; the 65th
               column of v_aug is 1 so the same matmul yields Z; normalize
               by 1/Z straight out of PSUM (DVE recip + ACT copy-scale).

The per-head work is emitted as a 4-stage software pipeline
(proj h | stats h-1 | scores+exp h-2 | context+normalize h-3) with the
stages' instructions interleaved step-by-step so the in-order engine
queues (PE / DVE / ACT) always have independent work.
All f32 matmuls run in float32r (fast PE mode, fp32 PSUM accumulation).
"""
import sys

sys.path.insert(0, "/opt/trn_rl_repo")

import numpy as np
import concourse.bacc as bacc
import concourse.mybir as mybir
import concourse.tile as tile
from concourse.bass_utils import run_bass_kernel_spmd

F32 = mybir.dt.float32
F32R = mybir.dt.float32r
BF16 = mybir.dt.bfloat16
EXP = mybir.ActivationFunctionType.Exp
COPY = mybir.ActivationFunctionType.Copy

HD = 64  # head dim (fixed)


def build_module(T, H, NH):
    """One-core program; run SPMD on 8 cores with per-core batch slices."""
    NT = T // 128      # token tiles
    NHT = H // 128     # hidden-dim tiles
    QC = min(512, T)   # moving chunk (>=256 keeps f32r at full rate)
    NQC = T // QC

    nc = bacc.Bacc("TRN2", target_bir_lowering=False, debug=False, num_devices=8)

    hidden = nc.dram_tensor("hidden", [T, H], F32R, kind="ExternalInput").ap()
    w = nc.dram_tensor("w", [H, 3 * H], F32R, kind="ExternalInput").ap()
    mask_neg = nc.dram_tensor("mask_neg", [1, T], F32R, kind="ExternalInput").ap()
    ones_row = nc.dram_tensor("ones_row", [1, T], F32R, kind="ExternalInput").ap()
    neg_row = nc.dram_tensor("neg_row", [1, T], F32R, kind="ExternalInput").ap()
    qkb = nc.dram_tensor("qkb", [1, 128 * NH], F32R, kind="ExternalInput").ap()
    vb = nc.dram_tensor("vb", [1, H], F32R, kind="ExternalInput").ap()
    ident_r = nc.dram_tensor("ident_r", [128, 128], F32R, kind="ExternalInput").ap()
    out = nc.dram_tensor("out", [T, H], F32, kind="ExternalOutput").ap()

    out_r = out.rearrange("(qt p) (h d) -> p qt h d", p=128, d=HD)

    with tile.TileContext(nc) as tc:
        with tc.tile_pool(name="persist", bufs=1) as persist, tc.tile_pool(
            name="work", bufs=2
        ) as work, tc.tile_pool(name="e8p", bufs=20) as e8p, tc.tile_pool(
            name="cmp", bufs=2
        ) as cmp, tc.tile_pool(name="outp", bufs=2) as outp, tc.tile_pool(
            name="rzp", bufs=3
        ) as rzp, tc.tile_pool(
            name="psP", bufs=1, space="PSUM"
        ) as psP, tc.tile_pool(name="psS", bufs=1, space="PSUM") as psS, tc.tile_pool(
            name="psE", bufs=1, space="PSUM"
        ) as psE, tc.tile_pool(name="psC", bufs=1, space="PSUM") as psC:
            # ---- static constants ----
            idr = persist.tile([128, 128], F32R, tag="idr")
            nc.sync.dma_start(out=idr, in_=ident_r)
            onesr = persist.tile([1, T], F32R, tag="onesr")
            nc.sync.dma_start(out=onesr, in_=ones_row)
            qkbt = persist.tile([1, 128 * NH], F32R, tag="qkbt")
            nc.sync.dma_start(out=qkbt, in_=qkb)
            vbt = persist.tile([1, H], F32R, tag="vbt")
            nc.sync.dma_start(out=vbt, in_=vb)

            # triple-buffered q/k aux buffers (head h uses slot h%3):
            # qaux rows: 0:64 q, 64 ones (static), 65 c=+max (per head)
            # kaux rows: 0:64 k, 64 mask*-1e4 (static), 65 -1 (static)
            qauxs, kauxs = [], []
            for s in range(3):
                qa = persist.tile([66, T], F32R, tag=f"qaux{s}")
                ka = persist.tile([66, T], F32R, tag=f"kaux{s}")
                nc.sync.dma_start(out=qa[64:65, :], in_=ones_row)
                nc.sync.dma_start(out=ka[64:65, :], in_=mask_neg)
                nc.sync.dma_start(out=ka[65:66, :], in_=neg_row)
                qauxs.append(qa)
                kauxs.append(ka)

            # ---- phase 0: hT[p, ht, t] = hidden[t, ht*128+p] ----
            hT = persist.tile([128, NHT, T], F32R, tag="hT")
            for t in range(NT):
                hid = work.tile([128, H], F32R, tag="hid")
                nc.sync.dma_start(out=hid, in_=hidden[t * 128 : (t + 1) * 128, :])
                pool = psS if t % 2 == 0 else psE
                xpt = pool.tile([128, NHT, 128], F32R, tag="t")
                for hb in range(NHT):
                    nc.tensor.transpose(
                        xpt[:, hb, :], hid[:, hb * 128 : (hb + 1) * 128], idr[:]
                    )
                nc.vector.tensor_copy(hT[:, :, t * 128 : (t + 1) * 128], xpt[:])

            # ---- phase 1: v_aug[p, kt, h, 0:64] = v proj + bias; [.., 64] = 1 ----
            wv = persist.tile([128, NHT, H], F32R, tag="wv")
            for ht in range(NHT):
                wsl = w[ht * 128 : (ht + 1) * 128, :].rearrange(
                    "p (h three d) -> p h three d", three=3, d=HD
                )
                nc.sync.dma_start(
                    out=wv[:, ht, :].rearrange("p (h d) -> p h d", d=HD),
                    in_=wsl[:, :, 2, :],
                )
            v_aug = persist.tile([128, NT, NH, HD + 1], BF16, tag="v_aug")
            nc.vector.memset(v_aug[:, :, :, HD : HD + 1], 1.0)
            VW = min(512, H)
            NVH = VW // HD
            for t in range(NT):
                for half in range(H // VW):
                    pool = psP if (t * (H // VW) + half) % 2 == 0 else psS
                    vp = pool.tile([128, VW], F32, tag="t")
                    for ht in range(NHT):
                        nc.tensor.matmul(
                            vp[:],
                            hT[:, ht, t * 128 : (t + 1) * 128],
                            wv[:, ht, half * VW : (half + 1) * VW],
                            start=(ht == 0),
                            stop=False,
                        )
                    # bias: out[t, f] += 1 * vb[f]
                    nc.tensor.matmul(
                        vp[:],
                        onesr[0:1, t * 128 : (t + 1) * 128],
                        vbt[0:1, half * VW : (half + 1) * VW],
                        start=False,
                        stop=True,
                    )
                    nc.vector.tensor_copy(
                        v_aug[:, t, half * NVH : (half + 1) * NVH, 0:HD],
                        vp[:].rearrange("p (h d) -> p h d", d=HD),
                    )

            # ---- per-head attention: 4-stage software pipeline ----
            state = {}  # head -> dict with live tiles

            def proj_ops(h):
                """Return (closures) emitting the qk projection matmuls."""
                st = state[h]
                wqk = work.tile([128, NHT, 128], F32R, tag="wqk")
                nc.sync.dma_start(
                    out=wqk,
                    in_=w[:, h * 3 * HD : h * 3 * HD + 128].rearrange(
                        "(ht p) f -> p ht f", p=128
                    ),
                )
                qkp = psP.tile([128, T], F32, tag="t")
                st["qkp"] = qkp
                ops = []
                for qc in range(NQC):
                    for ht in range(NHT):
                        ops.append(
                            lambda qc=qc, ht=ht: nc.tensor.matmul(
                                qkp[:, qc * QC : (qc + 1) * QC],
                                wqk[:, ht, :],
                                hT[:, ht, qc * QC : (qc + 1) * QC],
                                start=(ht == 0),
                                stop=False,
                            )
                        )
                    ops.append(
                        lambda qc=qc: nc.tensor.matmul(
                            qkp[:, qc * QC : (qc + 1) * QC],
                            qkbt[0:1, h * 128 : (h + 1) * 128],
                            onesr[0:1, qc * QC : (qc + 1) * QC],
                            start=False,
                            stop=True,
                        )
                    )
                return ops

            NSTEP = NT  # steps per iteration
            for it in range(NH + 3):
                hP, hS, hE, hC = it, it - 1, it - 2, it - 3

                doP = hP < NH
                doS = 0 <= hS < NH
                doE = 0 <= hE < NH
                doC = 0 <= hC < NH

                pops = []
                if doP:
                    state[hP] = {}
                    pops = proj_ops(hP)
                if doS:
                    stS = state[hS]
                    stS["cmat"] = cmp.tile([128, NT], F32R, tag="cmat", name="cmat")
                if doE:
                    stE = state[hE]
                    stE["e8s"] = []
                if doC:
                    stC = state[hC]
                    stC["ost"] = outp.tile([128, NT, HD], F32, tag="ost", name="ost")
                    stC["ctq"] = psC.tile([128, NT, 128], F32, tag="cq", name="ctq")

                ppos = 0
                nper = (len(pops) + NSTEP - 1) // NSTEP if pops else 0
                for i in range(NSTEP):
                    # stats(hS) step: scores [q-tile i, all k] -> row max
                    if doS:
                        qa, ka = qauxs[hS % 3], kauxs[hS % 3]
                        smx = psS.tile([128, T], F32, tag="t")
                        for qc in range(NQC):
                            nc.tensor.matmul(
                                smx[:, qc * QC : (qc + 1) * QC],
                                qa[0:65, i * 128 : (i + 1) * 128],
                                ka[0:65, qc * QC : (qc + 1) * QC],
                                start=True,
                                stop=True,
                            )
                        nc.vector.reduce_max(
                            stS["cmat"][:, i : i + 1], smx[:],
                            axis=mybir.AxisListType.X,
                        )
                    # proj(hP) chunk
                    for _ in range(nper):
                        if ppos < len(pops):
                            pops[ppos]()
                            ppos += 1
                    # pass2(hE) step: scores [k-tile i, all q] -> exp -> bf16
                    if doE:
                        qa, ka = qauxs[hE % 3], kauxs[hE % 3]
                        sp = psE.tile([128, T], F32, tag="t")
                        for qc in range(NQC):
                            nc.tensor.matmul(
                                sp[:, qc * QC : (qc + 1) * QC],
                                ka[0:66, i * 128 : (i + 1) * 128],
                                qa[0:66, qc * QC : (qc + 1) * QC],
                                start=True,
                                stop=True,
                            )
                        e8 = e8p.tile([128, T], BF16, tag="e8")
                        nc.scalar.activation(out=e8[:], in_=sp[:], func=EXP, scale=8.0)
                        stE["e8s"].append(e8)
                    # ctx+norm(hC) step: context for q-tile i over all k tiles
                    if doC:
                        ctq = stC["ctq"]
                        for kt in range(NT):
                            nc.tensor.matmul(
                                ctq[:, i, 0 : HD + 1],
                                stC["e8s"][kt][:, i * 128 : (i + 1) * 128],
                                v_aug[:, kt, hC, :],
                                start=(kt == 0),
                                stop=(kt == NT - 1),
                            )
                        rz = rzp.tile([128, 1], F32, tag="rz")
                        nc.vector.reciprocal(rz[:], ctq[:, i, HD : HD + 1])
                        nc.scalar.activation(
                            out=stC["ost"][:, i, :],
                            in_=ctq[:, i, 0:HD],
                            func=COPY,
                            scale=rz[:],
                        )

                # emit any leftover proj ops
                while ppos < len(pops):
                    pops[ppos]()
                    ppos += 1

                # stats(hS) epilogue: c row -> qaux[65]
                if doS:
                    ctile = psS.tile([NT, 128], F32R, tag="t")
                    nc.tensor.transpose(ctile[:], stS["cmat"][:], idr[:])
                    ctr = work.tile([NT, 128], F32R, tag="ctr")
                    nc.vector.tensor_copy(ctr[:], ctile[:])
                    nc.sync.dma_start(out=qauxs[hS % 3][65:66, :], in_=ctr[:])

                # proj(hP) epilogue: q/k to SBUF aux buffers
                if doP:
                    qa, ka = qauxs[hP % 3], kauxs[hP % 3]
                    nc.vector.tensor_copy(qa[0:64, :], state[hP]["qkp"][0:64, :])
                    nc.vector.tensor_copy(ka[0:64, :], state[hP]["qkp"][64:128, :])

                # ctx(hC) epilogue: store
                if doC:
                    nc.sync.dma_start(out=out_r[:, :, hC, :], in_=stC["ost"])
                    del state[hC]

    nc.compile()
    return nc


_module_cache = {}


def _get_module(T, H, NH):
    key = (T, H, NH)
    if key not in _module_cache:
        _module_cache[key] = build_module(T, H, NH)
    return _module_cache[key]


def run_sharded(hidden_states, attention_mask, w_qkv, b_qkv, trace=False):
    B, T, H = hidden_states.shape
    NH = H // HD
    nc = _get_module(T, H, NH)

    w_np = np.ascontiguousarray(w_qkv.astype(np.float32))
    b_np = np.asarray(b_qkv, dtype=np.float32)
    # qkb[h*128 + p] = b[h*192 + p]  (q bias 0:64, k bias 64:128 per head)
    qkb = np.empty((1, 128 * NH), np.float32)
    vb = np.empty((1, H), np.float32)
    for h in range(NH):
        qkb[0, h * 128 : (h + 1) * 128] = b_np[h * 3 * HD : h * 3 * HD + 128]
        vb[0, h * HD : (h + 1) * HD] = b_np[h * 3 * HD + 2 * HD : h * 3 * HD + 3 * HD]
    ones_row = np.ones((1, T), np.float32)
    neg_row = np.full((1, T), -1.0, np.float32)
    ident = np.eye(128, dtype=np.float32)

    in_maps = []
    for b in range(B):
        m = np.asarray(attention_mask[b]).reshape(-1).astype(np.float32)
        in_maps.append(
            dict(
                hidden=np.ascontiguousarray(hidden_states[b].astype(np.float32)),
                w=w_np,
                mask_neg=(m * np.float32(-10000.0)).reshape(1, T),
                ones_row=ones_row,
                neg_row=neg_row,
                qkb=qkb,
                vb=vb,
                ident_r=ident,
            )
        )
    res = run_bass_kernel_spmd(nc, in_maps, core_ids=list(range(B)), trace=trace)
    return np.stack([res.results[b]["out"] for b in range(B)]), res


def kernel(hidden_states, attention_mask, w_qkv, b_qkv):
    out, _ = run_sharded(
        np.asarray(hidden_states),
        np.asarray(attention_mask),
        np.asarray(w_qkv),
        np.asarray(b_qkv),
    )
    return out.astype(np.float32)
